# revision 2
# baseline (speedup 1.0000x reference)
"""CompGCN (2-layer) Trainium2 kernel, 8-core SPMD — transfer-optimized v2.

The measured bottleneck in this environment is the axon tunnel (~70MB/s up,
~125MB/s down), so v2 minimizes wire bytes:
 - x is uploaded SHARDED (each core gets only its [NPC, D] slice, bf16);
   the full norm-prescaled gather table is assembled on-device via AllGather
   (layer 2 already worked this way; layer 1 now does too).
 - Edge metadata ships non-replicated: int16 gather indices as [16, cols]
   (the required 128-partition replication is done on-device with three
   doubling SBUF->SBUF DMAs) and uint8 dst-slot codes.
 - Weights are host-folded (loop_w*loop_rel/3, rel@w_rel, BN affine) and
   shipped bf16; all gathers/matmuls run in bf16 (f32 PSUM accumulate).
 - Output is returned as bf16 [NPC, D] per core and upcast on host.

Compute strategy is unchanged from v1: node-range sharding with dst-sorted
edges, per-128-node-block scatter via one-hot matmuls in PSUM, norm[dst]
folded into the PSUM->SBUF copy, 3-matmul node update + fused BN/tanh.
"""

import math
import os
import numpy as np
import ml_dtypes

N, E, D, R, L = 50000, 800000, 128, 16, 2
SPLIT = 32768
BN_EPS = 1e-5
P = 128
M = 8
NPC = N // M                    # 6250 nodes per core
NBLK = (NPC + P - 1) // P       # 49
LASTR = NPC - (NBLK - 1) * P    # 106 rows in last block

BF16 = ml_dtypes.bfloat16

_CACHE = {}
LAST_RESULTS = None


def _preprocess(src, dst, edge_type):
    src = np.asarray(src).astype(np.int64, copy=False)
    dst = np.asarray(dst).astype(np.int64, copy=False)
    et = np.asarray(edge_type).astype(np.int64, copy=False)
    deg = np.bincount(dst, minlength=N).astype(np.float32)

    half = E // 2
    passes = []
    maxL = maxH = 0
    for sl in (slice(0, half), slice(half, E)):
        s, d, t = src[sl], dst[sl], et[sl]
        core = d // NPC
        rem = d - core * NPC
        blk = rem >> 7
        slot = rem & 127
        hi = (s >= SPLIT).astype(np.int64)
        key = ((core * NBLK + blk) << 1) | hi
        order = np.argsort(key.astype(np.int16), kind="stable")
        ks = key[order]
        counts = np.bincount(key, minlength=M * NBLK * 2)
        starts = np.zeros_like(counts)
        starts[1:] = np.cumsum(counts)[:-1]
        pos = np.arange(len(ks), dtype=np.int64) - starts[ks]
        passes.append((s[order], t[order], slot[order], ks, pos))
        maxL = max(maxL, int(counts[0::2].max()))
        maxH = max(maxH, int(counts[1::2].max()))
    tl = -(-maxL // P)
    th = -(-maxH // P)
    tpb = tl + th
    WB = (tl + th + tpb) * 8        # int16 cols per block in meta
    COLS = NBLK * WB
    KW = NBLK * tpb

    meta = np.zeros((M, 2, 16, COLS), np.int16)
    slotu = np.full((M, 2, P, KW), 255, np.uint8)
    for pi, (s_s, t_s, sl_s, ks, pos) in enumerate(passes):
        cb = ks >> 1
        hi = ks & 1
        core_s = cb // NBLK
        blk_s = cb % NBLK
        base = blk_s * WB
        # x-table index: position pos within the (blk, hi) bucket
        col = base + hi * (tl * 8) + (pos >> 4)
        meta[core_s, pi, pos & 15, col] = (s_s - hi * SPLIT).astype(np.int16)
        # rel index + slot at flat position within the block's tpb*P range
        fl = hi * (tl * P) + pos
        colr = base + (tl + th) * 8 + (fl >> 4)
        meta[core_s, pi, fl & 15, colr] = t_s.astype(np.int16)
        slotu[core_s, pi, fl & 127, blk_s * tpb + (fl >> 7)] = sl_s.astype(np.uint8)

    degp = np.zeros((M, NBLK * P), np.float32)
    degp[:, :NPC] = deg.reshape(M, NPC)
    deg_own = np.ascontiguousarray(degp.reshape(M, NBLK, P).transpose(0, 2, 1))
    return meta, slotu, deg_own, tl, th


def _fold_weights(inputs):
    f32 = np.float32
    in_w = np.asarray(inputs["in_w"], f32)
    out_w = np.asarray(inputs["out_w"], f32)
    loop_w = np.asarray(inputs["loop_w"], f32)
    w_rel = np.asarray(inputs["w_rel"], f32)
    loop_rel = np.asarray(inputs["loop_rel"], f32)
    lw3 = [loop_w[l] * loop_rel[l, 0][:, None] / 3.0 for l in range(L)]
    wts = np.ascontiguousarray(
        np.stack([in_w[0], out_w[0], lw3[0], in_w[1], out_w[1], lw3[1]]).astype(BF16))
    r1 = np.asarray(inputs["init_rel"], f32)[:R]
    r2 = r1 @ w_rel[0]
    rels = np.ascontiguousarray(np.stack([r1, r2]).astype(BF16))
    gam = np.asarray(inputs["bn_gamma"], f32)
    bet = np.asarray(inputs["bn_beta"], f32)
    bia = np.asarray(inputs["bias"], f32)
    bns = gam / np.sqrt(1.0 + BN_EPS)
    beff = bia * bns + bet
    misc = np.ascontiguousarray(np.stack([bns[0], beff[0], bns[1], beff[1]]))
    return wts, rels, misc


def _build_nc(tl, th):
    tpb = tl + th
    WB = (tl + th + tpb) * 8
    COLS = NBLK * WB
    KW = NBLK * tpb

    import concourse.bass as bass
    import concourse.tile as tile
    from concourse import bacc, mybir

    f32 = mybir.dt.float32
    bf = mybir.dt.bfloat16
    i16 = mybir.dt.int16
    u8 = mybir.dt.uint8
    Alu = mybir.AluOpType
    Act = mybir.ActivationFunctionType

    nc = bacc.Bacc("TRN2", target_bir_lowering=False, debug=False,
                   num_devices=M)

    x_ext = nc.dram_tensor("xb", [NPC, D], bf, kind="ExternalInput")
    meta_ext = nc.dram_tensor("meta", [2, 16, COLS], i16, kind="ExternalInput")
    slot_ext = nc.dram_tensor("slotu", [2, P, KW], u8, kind="ExternalInput")
    deg_ext = nc.dram_tensor("dego", [P, NBLK], f32, kind="ExternalInput")
    wts_ext = nc.dram_tensor("wts", [6, D, D], bf, kind="ExternalInput")
    rels_ext = nc.dram_tensor("rels", [2, R, D], bf, kind="ExternalInput")
    misc_ext = nc.dram_tensor("misc", [4, D], f32, kind="ExternalInput")
    out_ext = nc.dram_tensor("xout", [NPC, D], bf, kind="ExternalOutput")

    with tile.TileContext(nc) as tc:
        from contextlib import ExitStack
        with ExitStack() as ctx:
            cpool = ctx.enter_context(tc.tile_pool(name="const", bufs=1))
            big = ctx.enter_context(tc.tile_pool(name="big", bufs=1))
            gp = ctx.enter_context(tc.tile_pool(name="gather", bufs=3))
            sp = ctx.enter_context(tc.tile_pool(name="small", bufs=3))
            dp = ctx.enter_context(tc.tile_pool(name="dram", bufs=1, space="DRAM"))
            ps_agg = ctx.enter_context(tc.tile_pool(name="ps_agg", bufs=2, space="PSUM"))
            ps_h = ctx.enter_context(tc.tile_pool(name="ps_h", bufs=2, space="PSUM"))
            ps_t = ctx.enter_context(tc.tile_pool(name="ps_t", bufs=2, space="PSUM"))

            from concourse.library_config import mlp as _mlp_lib
            nc.gpsimd.load_library(_mlp_lib)

            ag_in = dp.tile([NPC, D], bf, name="ag_in")
            ag_out1 = dp.tile([N, D], bf, name="ag_out1", addr_space="Shared")
            ag_out2 = dp.tile([N, D], bf, name="ag_out2", addr_space="Shared")

            # ---------- constants built on device ----------
            iota_t = cpool.tile([P, tpb * P], bf, name="iota_t")
            nc.gpsimd.iota(iota_t[:].rearrange("p (k s) -> p k s", s=P),
                           [[0, tpb], [1, P]], channel_multiplier=0,
                           allow_small_or_imprecise_dtypes=True)
            ii = sp.tile([P, P], f32, tag="identtmp", bufs=1)
            nc.gpsimd.iota(ii[:], [[1, P]], channel_multiplier=0,
                           allow_small_or_imprecise_dtypes=True)
            jj = sp.tile([P, P], f32, tag="identtmp2", bufs=1)
            nc.gpsimd.iota(jj[:], [[0, P]], channel_multiplier=1,
                           allow_small_or_imprecise_dtypes=True)
            ident = cpool.tile([P, P], f32, name="ident")
            nc.vector.tensor_tensor(out=ident[:], in0=ii[:], in1=jj[:],
                                    op=Alu.is_equal)
            identb = cpool.tile([P, P], bf, name="identb")
            nc.vector.tensor_copy(out=identb[:], in_=ident[:])

            # ---------- small inputs ----------
            wt_sb = cpool.tile([D, 6 * D], bf, name="wt_sb")
            nc.sync.dma_start(out=wt_sb[:].rearrange("r (k c) -> r k c", c=D),
                              in_=wts_ext[:, :, :].rearrange("k r c -> r k c"))
            mvec = cpool.tile([D, 4], f32, name="mvec")
            nc.sync.dma_start(out=mvec[:], in_=misc_ext[:, :].rearrange("k d -> d k"))

            meta_sb = cpool.tile([P, 2 * COLS], i16, name="meta_sb")
            nc.sync.dma_start(out=meta_sb[0:16, 0:COLS], in_=meta_ext[0])
            nc.sync.dma_start(out=meta_sb[0:16, COLS:2 * COLS], in_=meta_ext[1])
            nc.sync.dma_start(out=meta_sb[16:32, :], in_=meta_sb[0:16, :])
            nc.sync.dma_start(out=meta_sb[32:64, :], in_=meta_sb[0:32, :])
            nc.sync.dma_start(out=meta_sb[64:128, :], in_=meta_sb[0:64, :])

            slot_bf = cpool.tile([P, 2 * KW], bf, name="slot_bf")
            for pi in range(2):
                su = sp.tile([P, KW], u8, tag="slotload")
                nc.sync.dma_start(out=su[:], in_=slot_ext[pi])
                nc.vector.tensor_copy(out=slot_bf[:, pi * KW:(pi + 1) * KW],
                                      in_=su[:])

            # ---------- norm from degrees ----------
            dg = sp.tile([P, NBLK], f32, tag="degload", bufs=1)
            nc.sync.dma_start(out=dg[:], in_=deg_ext[:, :])
            t1 = sp.tile([P, NBLK], f32, tag="normtmp", bufs=1)
            nc.vector.tensor_scalar(out=t1[:], in0=dg[:], scalar1=1.0,
                                    scalar2=None, op0=Alu.max)
            nc.vector.reciprocal(t1[:], t1[:])
            nc.scalar.sqrt(t1[:], t1[:])
            msk = sp.tile([P, NBLK], f32, tag="normmask", bufs=1)
            nc.vector.tensor_scalar(out=msk[:], in0=dg[:], scalar1=0.0,
                                    scalar2=None, op0=Alu.is_gt)
            norm_own = cpool.tile([P, NBLK], f32, name="norm_own")
            nc.vector.tensor_tensor(out=norm_own[:], in0=t1[:], in1=msk[:],
                                    op=Alu.mult)

            # norm_bcast[d, b*128+s] = norm_own[s, b]
            norm_bcast = big.tile([P, NBLK * P], bf, name="norm_bcast")
            for b in range(NBLK):
                pt = ps_t.tile([P, P], f32)
                nc.tensor.transpose(pt[:], norm_own[:, b:b + 1].to_broadcast([P, P]),
                                    ident[:])
                nc.vector.tensor_copy(out=norm_bcast[:, b * P:(b + 1) * P], in_=pt[:])

            # ---------- own x: transpose into x_curT, normalized copy to ag_in ----------
            x_curT = big.tile([P, NBLK * P], bf, name="x_curT")
            for b in range(NBLK):
                rows = P if b < NBLK - 1 else LASTR
                xbuf = sp.tile([P, D], bf, tag="xload")
                if rows < P:
                    nc.vector.memset(xbuf[:], 0.0)
                nc.sync.dma_start(out=xbuf[:rows, :],
                                  in_=x_ext[b * P:b * P + rows, :])
                pt = ps_t.tile([P, P], bf)
                nc.tensor.transpose(pt[:], xbuf[:], identb[:])
                nc.vector.tensor_copy(out=x_curT[:, b * P:(b + 1) * P], in_=pt[:])
                xn = sp.tile([P, D], bf, tag="xnorm")
                nc.vector.tensor_scalar(out=xn[:], in0=xbuf[:],
                                        scalar1=norm_own[:, b:b + 1],
                                        scalar2=None, op0=Alu.mult)
                nc.sync.dma_start(out=ag_in[b * P:b * P + rows, :],
                                  in_=xn[:rows, :])
            nc.gpsimd.collective_compute(
                "AllGather", Alu.bypass,
                replica_groups=[list(range(M))],
                ins=[ag_in[:].opt()], outs=[ag_out1[:].opt()])

            aggT = [big.tile([P, NBLK * P], bf, name=f"aggT{pi}") for pi in range(2)]

            # ================= layers =================
            for l in range(L):
                ag_out = ag_out1 if l == 0 else ag_out2
                for pi in range(2):
                    for b in range(NBLK):
                        cb = pi * COLS + b * WB
                        xg = gp.tile([P, tpb * P], bf, tag="xg")
                        nc.gpsimd.dma_gather(
                            out_ap=xg[:, :tl * P].rearrange(
                                "p (k d) -> p k d", d=D),
                            in_ap=ag_out[:, :],
                            idxs_ap=meta_sb[:, cb:cb + tl * 8],
                            num_idxs=tl * P, num_idxs_reg=tl * P,
                            elem_size=D, single_packet=False)
                        nc.gpsimd.dma_gather(
                            out_ap=xg[:, tl * P:].rearrange(
                                "p (k d) -> p k d", d=D),
                            in_ap=ag_out[SPLIT:, :],
                            idxs_ap=meta_sb[:, cb + tl * 8:cb + (tl + th) * 8],
                            num_idxs=th * P, num_idxs_reg=th * P,
                            elem_size=D, single_packet=False)
                        rg = gp.tile([P, tpb * P], bf, tag="rg")
                        nc.gpsimd.dma_gather(
                            out_ap=rg[:].rearrange("p (k d) -> p k d", d=D),
                            in_ap=rels_ext[l],
                            idxs_ap=meta_sb[:, cb + (tl + th) * 8:cb + WB],
                            num_idxs=tpb * P, num_idxs_reg=tpb * P,
                            elem_size=D, single_packet=False)
                        nc.vector.tensor_tensor(out=xg[:], in0=xg[:], in1=rg[:],
                                                op=Alu.mult)
                        oh = gp.tile([P, tpb * P], bf, tag="oh")
                        nc.vector.tensor_tensor(
                            out=oh[:], in0=iota_t[:],
                            in1=slot_bf[:, pi * KW + b * tpb:pi * KW + (b + 1) * tpb]
                                .to_broadcast([P, tpb, P]),
                            op=Alu.is_equal)
                        agp = ps_agg.tile([P, P], f32)
                        for j in range(tpb):
                            nc.tensor.matmul(agp[:],
                                             lhsT=xg[:, j * P:(j + 1) * P],
                                             rhs=oh[:, j * P:(j + 1) * P],
                                             start=(j == 0), stop=(j == tpb - 1))
                        nc.vector.tensor_tensor(
                            out=aggT[pi][:, b * P:(b + 1) * P], in0=agp[:],
                            in1=norm_bcast[:, b * P:(b + 1) * P], op=Alu.mult)

                # node update
                for b in range(NBLK):
                    bs = slice(b * P, (b + 1) * P)
                    rows = P if b < NBLK - 1 else LASTR
                    w0 = slice((3 * l) * D, (3 * l + 1) * D)
                    w1 = slice((3 * l + 1) * D, (3 * l + 2) * D)
                    w2 = slice((3 * l + 2) * D, (3 * l + 3) * D)
                    hp = ps_h.tile([P, P], f32)
                    nc.tensor.matmul(hp[:], lhsT=wt_sb[:, w0],
                                     rhs=aggT[0][:, bs], start=True, stop=False)
                    nc.tensor.matmul(hp[:], lhsT=wt_sb[:, w1],
                                     rhs=aggT[1][:, bs], start=False, stop=False)
                    nc.tensor.matmul(hp[:], lhsT=wt_sb[:, w2],
                                     rhs=x_curT[:, bs], start=False, stop=True)
                    xc = sp.tile([P, P], f32, tag="xact")
                    nc.scalar.activation(out=xc[:], in_=hp[:], func=Act.Tanh,
                                         bias=mvec[:, 2 * l + 1:2 * l + 2],
                                         scale=mvec[:, 2 * l:2 * l + 1])
                    pt = ps_t.tile([P, P], f32)
                    nc.tensor.transpose(pt[:], xc[:], ident[:])
                    if l == 0:
                        nc.vector.tensor_copy(out=x_curT[:, bs], in_=xc[:])
                        xn = sp.tile([P, P], bf, tag="xupd")
                        nc.vector.tensor_scalar(out=xn[:], in0=pt[:],
                                                scalar1=norm_own[:, b:b + 1],
                                                scalar2=None, op0=Alu.mult)
                        nc.sync.dma_start(out=ag_in[b * P:b * P + rows, :],
                                          in_=xn[:rows, :])
                    else:
                        xo = sp.tile([P, P], bf, tag="xout")
                        nc.vector.tensor_copy(out=xo[:], in_=pt[:])
                        nc.sync.dma_start(out=out_ext[b * P:b * P + rows, :],
                                          in_=xo[:rows, :])
                if l == 0:
                    nc.gpsimd.collective_compute(
                        "AllGather", Alu.bypass,
                        replica_groups=[list(range(M))],
                        ins=[ag_in[:].opt()], outs=[ag_out2[:].opt()])
    nc.compile()
    return nc


def kernel(**inputs):
    global LAST_RESULTS
    meta, slotu, deg_own, tl, th = _preprocess(
        inputs["src"], inputs["dst"], inputs["edge_type"])
    if (tl, th) not in _CACHE:
        _CACHE[(tl, th)] = _build_nc(tl, th)
    nc = _CACHE[(tl, th)]

    wts, rels, misc = _fold_weights(inputs)
    xb = np.ascontiguousarray(np.asarray(inputs["x"], np.float32).astype(BF16))
    in_maps = []
    for c in range(M):
        in_maps.append(dict(
            xb=xb[c * NPC:(c + 1) * NPC],
            meta=meta[c], slotu=slotu[c], dego=deg_own[c],
            wts=wts, rels=rels, misc=misc,
        ))

    from concourse.bass_utils import run_bass_kernel_spmd
    trace = bool(int(os.environ.get("KERNEL_TRACE", "0")))
    res = run_bass_kernel_spmd(nc, in_maps, list(range(M)), trace=trace)
    LAST_RESULTS = res

    return np.concatenate(
        [res.results[c]["xout"].astype(np.float32) for c in range(M)], axis=0)


# revision 3
# speedup vs baseline: 1.5321x; 1.5321x over previous
"""CompGCN (2-layer) Trainium2 kernel, 8-core SPMD — transfer-optimized v2.

The measured bottleneck in this environment is the axon tunnel (~70MB/s up,
~125MB/s down), so v2 minimizes wire bytes:
 - x is uploaded SHARDED (each core gets only its [NPC, D] slice, bf16);
   the full norm-prescaled gather table is assembled on-device via AllGather
   (layer 2 already worked this way; layer 1 now does too).
 - Edge metadata ships non-replicated: int16 gather indices as [16, cols]
   (the required 128-partition replication is done on-device with three
   doubling SBUF->SBUF DMAs) and uint8 dst-slot codes.
 - Weights are host-folded (loop_w*loop_rel/3, rel@w_rel, BN affine) and
   shipped bf16; all gathers/matmuls run in bf16 (f32 PSUM accumulate).
 - Output is returned as bf16 [NPC, D] per core and upcast on host.

Compute strategy is unchanged from v1: node-range sharding with dst-sorted
edges, per-128-node-block scatter via one-hot matmuls in PSUM, norm[dst]
folded into the PSUM->SBUF copy, 3-matmul node update + fused BN/tanh.
"""

import math
import os
import numpy as np
import ml_dtypes

N, E, D, R, L = 50000, 800000, 128, 16, 2
SPLIT = 32768
BN_EPS = 1e-5
P = 128
M = 8
NPC = N // M                    # 6250 nodes per core
NBLK = (NPC + P - 1) // P       # 49
LASTR = NPC - (NBLK - 1) * P    # 106 rows in last block

BF16 = ml_dtypes.bfloat16

_CACHE = {}
LAST_RESULTS = None


def _preprocess(src, dst, edge_type):
    src = np.asarray(src).astype(np.int64, copy=False)
    dst = np.asarray(dst).astype(np.int64, copy=False)
    et = np.asarray(edge_type).astype(np.int64, copy=False)
    deg = np.bincount(dst, minlength=N).astype(np.float32)

    half = E // 2
    passes = []
    maxL = maxH = 0
    for sl in (slice(0, half), slice(half, E)):
        s, d, t = src[sl], dst[sl], et[sl]
        core = d // NPC
        rem = d - core * NPC
        blk = rem >> 7
        slot = rem & 127
        hi = (s >= SPLIT).astype(np.int64)
        key = ((core * NBLK + blk) << 1) | hi
        order = np.argsort(key.astype(np.int16), kind="stable")
        ks = key[order]
        counts = np.bincount(key, minlength=M * NBLK * 2)
        starts = np.zeros_like(counts)
        starts[1:] = np.cumsum(counts)[:-1]
        pos = np.arange(len(ks), dtype=np.int64) - starts[ks]
        passes.append((s[order], t[order], slot[order], ks, pos))
        maxL = max(maxL, int(counts[0::2].max()))
        maxH = max(maxH, int(counts[1::2].max()))
    tl = -(-maxL // P)
    th = -(-maxH // P)
    tpb = tl + th
    WB = (tl + th + tpb) * 8        # int16 cols per block in meta
    COLS = NBLK * WB
    KW = NBLK * tpb

    meta = np.zeros((M, 2, 16, COLS), np.int16)
    slotu = np.full((M, 2, P, KW), 255, np.uint8)
    for pi, (s_s, t_s, sl_s, ks, pos) in enumerate(passes):
        cb = ks >> 1
        hi = ks & 1
        core_s = cb // NBLK
        blk_s = cb % NBLK
        base = blk_s * WB
        # x-table index: position pos within the (blk, hi) bucket
        col = base + hi * (tl * 8) + (pos >> 4)
        meta[core_s, pi, pos & 15, col] = (s_s - hi * SPLIT).astype(np.int16)
        # rel index + slot at flat position within the block's tpb*P range
        fl = hi * (tl * P) + pos
        colr = base + (tl + th) * 8 + (fl >> 4)
        meta[core_s, pi, fl & 15, colr] = t_s.astype(np.int16)
        slotu[core_s, pi, fl & 127, blk_s * tpb + (fl >> 7)] = sl_s.astype(np.uint8)

    degp = np.zeros((M, NBLK * P), np.float32)
    degp[:, :NPC] = deg.reshape(M, NPC)
    deg_own = np.ascontiguousarray(degp.reshape(M, NBLK, P).transpose(0, 2, 1))
    return meta, slotu, deg_own, tl, th


def _fold_weights(inputs):
    f32 = np.float32
    in_w = np.asarray(inputs["in_w"], f32)
    out_w = np.asarray(inputs["out_w"], f32)
    loop_w = np.asarray(inputs["loop_w"], f32)
    w_rel = np.asarray(inputs["w_rel"], f32)
    loop_rel = np.asarray(inputs["loop_rel"], f32)
    lw3 = [loop_w[l] * loop_rel[l, 0][:, None] / 3.0 for l in range(L)]
    wts = np.ascontiguousarray(
        np.stack([in_w[0], out_w[0], lw3[0], in_w[1], out_w[1], lw3[1]]).astype(BF16))
    r1 = np.asarray(inputs["init_rel"], f32)[:R]
    r2 = r1 @ w_rel[0]
    rels = np.ascontiguousarray(np.stack([r1, r2]).astype(BF16))
    gam = np.asarray(inputs["bn_gamma"], f32)
    bet = np.asarray(inputs["bn_beta"], f32)
    bia = np.asarray(inputs["bias"], f32)
    bns = gam / np.sqrt(1.0 + BN_EPS)
    beff = bia * bns + bet
    misc = np.ascontiguousarray(np.stack([bns[0], beff[0], bns[1], beff[1]]))
    return wts, rels, misc


def _build_nc(tl, th, skip_edges=False):
    tpb = tl + th
    WB = (tl + th + tpb) * 8
    COLS = NBLK * WB
    KW = NBLK * tpb

    import concourse.bass as bass
    import concourse.tile as tile
    from concourse import bacc, mybir

    f32 = mybir.dt.float32
    bf = mybir.dt.bfloat16
    i16 = mybir.dt.int16
    u8 = mybir.dt.uint8
    Alu = mybir.AluOpType
    Act = mybir.ActivationFunctionType

    nc = bacc.Bacc("TRN2", target_bir_lowering=False, debug=False,
                   num_devices=M)

    x_ext = nc.dram_tensor("xb", [NPC, D], bf, kind="ExternalInput")
    meta_ext = nc.dram_tensor("meta", [2, 16, COLS], i16, kind="ExternalInput")
    slot_ext = nc.dram_tensor("slotu", [2, P, KW], u8, kind="ExternalInput")
    deg_ext = nc.dram_tensor("dego", [P, NBLK], f32, kind="ExternalInput")
    wts_ext = nc.dram_tensor("wts", [6, D, D], bf, kind="ExternalInput")
    rels_ext = nc.dram_tensor("rels", [2, R, D], bf, kind="ExternalInput")
    misc_ext = nc.dram_tensor("misc", [4, D], f32, kind="ExternalInput")
    out_ext = nc.dram_tensor("xout", [NPC, D], bf, kind="ExternalOutput")

    with tile.TileContext(nc) as tc:
        from contextlib import ExitStack
        with ExitStack() as ctx:
            cpool = ctx.enter_context(tc.tile_pool(name="const", bufs=1))
            big = ctx.enter_context(tc.tile_pool(name="big", bufs=1))
            gp = ctx.enter_context(tc.tile_pool(name="gather", bufs=3))
            sp = ctx.enter_context(tc.tile_pool(name="small", bufs=3))
            dp = ctx.enter_context(tc.tile_pool(name="dram", bufs=1, space="DRAM"))
            ps_agg = ctx.enter_context(tc.tile_pool(name="ps_agg", bufs=2, space="PSUM"))
            ps_h = ctx.enter_context(tc.tile_pool(name="ps_h", bufs=2, space="PSUM"))
            ps_t = ctx.enter_context(tc.tile_pool(name="ps_t", bufs=2, space="PSUM"))

            from concourse.library_config import mlp as _mlp_lib
            nc.gpsimd.load_library(_mlp_lib)

            ag_in = dp.tile([NPC, D], bf, name="ag_in")
            ag_out1 = dp.tile([N, D], bf, name="ag_out1", addr_space="Shared")
            ag_out2 = dp.tile([N, D], bf, name="ag_out2", addr_space="Shared")

            # ---------- constants built on device ----------
            iota_t = cpool.tile([P, tpb * P], bf, name="iota_t")
            nc.gpsimd.iota(iota_t[:].rearrange("p (k s) -> p k s", s=P),
                           [[0, tpb], [1, P]], channel_multiplier=0,
                           allow_small_or_imprecise_dtypes=True)
            ii = sp.tile([P, P], f32, tag="identtmp", bufs=1)
            nc.gpsimd.iota(ii[:], [[1, P]], channel_multiplier=0,
                           allow_small_or_imprecise_dtypes=True)
            jj = sp.tile([P, P], f32, tag="identtmp2", bufs=1)
            nc.gpsimd.iota(jj[:], [[0, P]], channel_multiplier=1,
                           allow_small_or_imprecise_dtypes=True)
            ident = cpool.tile([P, P], f32, name="ident")
            nc.vector.tensor_tensor(out=ident[:], in0=ii[:], in1=jj[:],
                                    op=Alu.is_equal)
            identb = cpool.tile([P, P], bf, name="identb")
            nc.vector.tensor_copy(out=identb[:], in_=ident[:])

            # ---------- small inputs ----------
            wt_sb = cpool.tile([D, 6 * D], bf, name="wt_sb")
            nc.sync.dma_start(out=wt_sb[:].rearrange("r (k c) -> r k c", c=D),
                              in_=wts_ext[:, :, :].rearrange("k r c -> r k c"))
            mvec = cpool.tile([D, 4], f32, name="mvec")
            nc.sync.dma_start(out=mvec[:], in_=misc_ext[:, :].rearrange("k d -> d k"))

            meta_sb = cpool.tile([P, 2 * COLS], i16, name="meta_sb")
            nc.sync.dma_start(out=meta_sb[0:16, 0:COLS], in_=meta_ext[0])
            nc.sync.dma_start(out=meta_sb[0:16, COLS:2 * COLS], in_=meta_ext[1])
            nc.sync.dma_start(out=meta_sb[16:32, :], in_=meta_sb[0:16, :])
            nc.sync.dma_start(out=meta_sb[32:64, :], in_=meta_sb[0:32, :])
            nc.sync.dma_start(out=meta_sb[64:128, :], in_=meta_sb[0:64, :])

            slot_bf = cpool.tile([P, 2 * KW], bf, name="slot_bf")
            for pi in range(2):
                su = sp.tile([P, KW], u8, tag="slotload")
                nc.sync.dma_start(out=su[:], in_=slot_ext[pi])
                nc.vector.tensor_copy(out=slot_bf[:, pi * KW:(pi + 1) * KW],
                                      in_=su[:])

            # ---------- norm from degrees ----------
            dg = sp.tile([P, NBLK], f32, tag="degload", bufs=1)
            nc.sync.dma_start(out=dg[:], in_=deg_ext[:, :])
            t1 = sp.tile([P, NBLK], f32, tag="normtmp", bufs=1)
            nc.vector.tensor_scalar(out=t1[:], in0=dg[:], scalar1=1.0,
                                    scalar2=None, op0=Alu.max)
            nc.vector.reciprocal(t1[:], t1[:])
            nc.scalar.sqrt(t1[:], t1[:])
            msk = sp.tile([P, NBLK], f32, tag="normmask", bufs=1)
            nc.vector.tensor_scalar(out=msk[:], in0=dg[:], scalar1=0.0,
                                    scalar2=None, op0=Alu.is_gt)
            norm_own = cpool.tile([P, NBLK], f32, name="norm_own")
            nc.vector.tensor_tensor(out=norm_own[:], in0=t1[:], in1=msk[:],
                                    op=Alu.mult)

            # norm_bcast[d, b*128+s] = norm_own[s, b]
            norm_bcast = big.tile([P, NBLK * P], bf, name="norm_bcast")
            for b in range(NBLK):
                pt = ps_t.tile([P, P], f32)
                nc.tensor.transpose(pt[:], norm_own[:, b:b + 1].to_broadcast([P, P]),
                                    ident[:])
                nc.vector.tensor_copy(out=norm_bcast[:, b * P:(b + 1) * P], in_=pt[:])

            # ---------- own x: transpose into x_curT, normalized copy to ag_in ----------
            x_curT = big.tile([P, NBLK * P], bf, name="x_curT")
            for b in range(NBLK):
                rows = P if b < NBLK - 1 else LASTR
                xbuf = sp.tile([P, D], bf, tag="xload")
                if rows < P:
                    nc.vector.memset(xbuf[:], 0.0)
                nc.sync.dma_start(out=xbuf[:rows, :],
                                  in_=x_ext[b * P:b * P + rows, :])
                pt = ps_t.tile([P, P], bf)
                nc.tensor.transpose(pt[:], xbuf[:], identb[:])
                nc.vector.tensor_copy(out=x_curT[:, b * P:(b + 1) * P], in_=pt[:])
                xn = sp.tile([P, D], bf, tag="xnorm")
                nc.vector.tensor_scalar(out=xn[:], in0=xbuf[:],
                                        scalar1=norm_own[:, b:b + 1],
                                        scalar2=None, op0=Alu.mult)
                nc.sync.dma_start(out=ag_in[b * P:b * P + rows, :],
                                  in_=xn[:rows, :])
            nc.gpsimd.collective_compute(
                "AllGather", Alu.bypass,
                replica_groups=[list(range(M))],
                ins=[ag_in[:].opt()], outs=[ag_out1[:].opt()])

            aggT = [big.tile([P, NBLK * P], bf, name=f"aggT{pi}") for pi in range(2)]

            # ================= layers =================
            if skip_edges:
                nc.vector.memset(aggT[0][:], 0.0)
                nc.vector.memset(aggT[1][:], 0.0)
            for l in range(L):
                ag_out = ag_out1 if l == 0 else ag_out2
                for pi in range(2):
                    if skip_edges:
                        break
                    for b in range(NBLK):
                        cb = pi * COLS + b * WB
                        xg = gp.tile([P, tpb * P], bf, tag="xg")
                        nc.gpsimd.dma_gather(
                            out_ap=xg[:, :tl * P].rearrange(
                                "p (k d) -> p k d", d=D),
                            in_ap=ag_out[:, :],
                            idxs_ap=meta_sb[:, cb:cb + tl * 8],
                            num_idxs=tl * P, num_idxs_reg=tl * P,
                            elem_size=D, single_packet=False)
                        nc.gpsimd.dma_gather(
                            out_ap=xg[:, tl * P:].rearrange(
                                "p (k d) -> p k d", d=D),
                            in_ap=ag_out[SPLIT:, :],
                            idxs_ap=meta_sb[:, cb + tl * 8:cb + (tl + th) * 8],
                            num_idxs=th * P, num_idxs_reg=th * P,
                            elem_size=D, single_packet=False)
                        rg = gp.tile([P, tpb * P], bf, tag="rg")
                        nc.gpsimd.dma_gather(
                            out_ap=rg[:].rearrange("p (k d) -> p k d", d=D),
                            in_ap=rels_ext[l],
                            idxs_ap=meta_sb[:, cb + (tl + th) * 8:cb + WB],
                            num_idxs=tpb * P, num_idxs_reg=tpb * P,
                            elem_size=D, single_packet=False)
                        nc.vector.tensor_tensor(out=xg[:], in0=xg[:], in1=rg[:],
                                                op=Alu.mult)
                        oh = gp.tile([P, tpb * P], bf, tag="oh")
                        nc.vector.tensor_tensor(
                            out=oh[:], in0=iota_t[:],
                            in1=slot_bf[:, pi * KW + b * tpb:pi * KW + (b + 1) * tpb]
                                .to_broadcast([P, tpb, P]),
                            op=Alu.is_equal)
                        agp = ps_agg.tile([P, P], f32)
                        for j in range(tpb):
                            nc.tensor.matmul(agp[:],
                                             lhsT=xg[:, j * P:(j + 1) * P],
                                             rhs=oh[:, j * P:(j + 1) * P],
                                             start=(j == 0), stop=(j == tpb - 1))
                        nc.vector.tensor_tensor(
                            out=aggT[pi][:, b * P:(b + 1) * P], in0=agp[:],
                            in1=norm_bcast[:, b * P:(b + 1) * P], op=Alu.mult)

                # node update
                for b in range(NBLK):
                    bs = slice(b * P, (b + 1) * P)
                    rows = P if b < NBLK - 1 else LASTR
                    w0 = slice((3 * l) * D, (3 * l + 1) * D)
                    w1 = slice((3 * l + 1) * D, (3 * l + 2) * D)
                    w2 = slice((3 * l + 2) * D, (3 * l + 3) * D)
                    hp = ps_h.tile([P, P], f32)
                    nc.tensor.matmul(hp[:], lhsT=wt_sb[:, w0],
                                     rhs=aggT[0][:, bs], start=True, stop=False)
                    nc.tensor.matmul(hp[:], lhsT=wt_sb[:, w1],
                                     rhs=aggT[1][:, bs], start=False, stop=False)
                    nc.tensor.matmul(hp[:], lhsT=wt_sb[:, w2],
                                     rhs=x_curT[:, bs], start=False, stop=True)
                    xc = sp.tile([P, P], f32, tag="xact")
                    nc.scalar.activation(out=xc[:], in_=hp[:], func=Act.Tanh,
                                         bias=mvec[:, 2 * l + 1:2 * l + 2],
                                         scale=mvec[:, 2 * l:2 * l + 1])
                    pt = ps_t.tile([P, P], f32)
                    nc.tensor.transpose(pt[:], xc[:], ident[:])
                    if l == 0:
                        nc.vector.tensor_copy(out=x_curT[:, bs], in_=xc[:])
                        xn = sp.tile([P, P], bf, tag="xupd")
                        nc.vector.tensor_scalar(out=xn[:], in0=pt[:],
                                                scalar1=norm_own[:, b:b + 1],
                                                scalar2=None, op0=Alu.mult)
                        nc.sync.dma_start(out=ag_in[b * P:b * P + rows, :],
                                          in_=xn[:rows, :])
                    else:
                        xo = sp.tile([P, P], bf, tag="xout")
                        nc.vector.tensor_copy(out=xo[:], in_=pt[:])
                        nc.sync.dma_start(out=out_ext[b * P:b * P + rows, :],
                                          in_=xo[:rows, :])
                if l == 0:
                    nc.gpsimd.collective_compute(
                        "AllGather", Alu.bypass,
                        replica_groups=[list(range(M))],
                        ins=[ag_in[:].opt()], outs=[ag_out2[:].opt()])
    nc.compile()
    return nc


_RUNNER = {}


def _get_runner(nc):
    """Build (once) a cached jitted SPMD executor for ``nc``.

    Same execution path as bass_utils.run_bass_kernel_spmd under axon
    (bass2jax._bass_exec_p -> neuronx_cc_hook -> PJRT), but the traced
    executable is cached across kernel() calls — avoiding per-call retrace /
    executable rebuild / NEFF reload — and the donated output buffers are
    created ON DEVICE (jnp.zeros under jit) instead of being uploaded.
    """
    key = id(nc)
    if key in _RUNNER:
        return _RUNNER[key]
    import jax
    import jax.numpy as jnp
    from jax.sharding import Mesh, PartitionSpec, NamedSharding
    from jax.experimental.shard_map import shard_map
    from concourse import bass2jax, mybir

    bass2jax.install_neuronx_cc_hook()
    assert nc.dbg_addr is None

    partition_name = (nc.partition_id_tensor.name
                      if nc.partition_id_tensor else None)
    in_names, out_names, out_avals = [], [], []
    for alloc in nc.m.functions[0].allocations:
        if not isinstance(alloc, mybir.MemoryLocationSet):
            continue
        name = alloc.memorylocations[0].name
        if alloc.kind == "ExternalInput":
            if name != partition_name:
                in_names.append(name)
        elif alloc.kind == "ExternalOutput":
            out_names.append(name)
            out_avals.append(jax.core.ShapedArray(
                tuple(alloc.tensor_shape), mybir.dt.np(alloc.dtype)))
    n_params = len(in_names)
    n_outs = len(out_avals)
    all_names = list(in_names) + list(out_names)
    if partition_name is not None:
        all_names.append(partition_name)

    def _body(*args):
        operands = list(args)
        if partition_name is not None:
            operands.append(bass2jax.partition_id_tensor())
        return tuple(bass2jax._bass_exec_p.bind(
            *operands,
            out_avals=tuple(out_avals),
            in_names=tuple(all_names),
            out_names=tuple(out_names),
            lowering_input_output_aliases=(),
            sim_require_finite=True,
            sim_require_nnan=True,
            nc=nc,
        ))

    devices = jax.devices()[:M]
    mesh = Mesh(np.asarray(devices), ("core",))
    in_specs = (PartitionSpec("core"),) * (n_params + n_outs)
    out_specs = (PartitionSpec("core"),) * n_outs
    donate = tuple(range(n_params, n_params + n_outs))
    sharded = jax.jit(
        shard_map(_body, mesh=mesh, in_specs=in_specs, out_specs=out_specs,
                  check_rep=False),
        donate_argnums=donate, keep_unused=True)

    zsh = tuple(NamedSharding(mesh, PartitionSpec("core")) for _ in out_avals)
    mkzeros = jax.jit(
        lambda: tuple(jnp.zeros((M * a.shape[0], *a.shape[1:]), a.dtype)
                      for a in out_avals),
        out_shardings=zsh)

    _RUNNER[key] = (sharded, mkzeros, in_names, out_names, n_params)
    return _RUNNER[key]


def kernel(**inputs):
    global LAST_RESULTS
    meta, slotu, deg_own, tl, th = _preprocess(
        inputs["src"], inputs["dst"], inputs["edge_type"])
    if (tl, th) not in _CACHE:
        _CACHE[(tl, th)] = _build_nc(tl, th)
    nc = _CACHE[(tl, th)]

    wts, rels, misc = _fold_weights(inputs)
    xb = np.ascontiguousarray(np.asarray(inputs["x"], np.float32).astype(BF16))
    in_maps = []
    for c in range(M):
        in_maps.append(dict(
            xb=xb[c * NPC:(c + 1) * NPC],
            meta=meta[c], slotu=slotu[c], dego=deg_own[c],
            wts=wts, rels=rels, misc=misc,
        ))

    try:
        sharded, mkzeros, in_names, out_names, n_params = _get_runner(nc)
        concat_in = [
            np.concatenate([m[name] for m in in_maps], axis=0)
            for name in in_names]
        outs = sharded(*concat_in, *mkzeros())
        LAST_RESULTS = None
        out = np.asarray(outs[out_names.index("xout")])
        return out.reshape(N, D).astype(np.float32)
    except Exception:
        if os.environ.get("KERNEL_NO_FALLBACK"):
            raise
        from concourse.bass_utils import run_bass_kernel_spmd
        res = run_bass_kernel_spmd(nc, in_maps, list(range(M)))
        LAST_RESULTS = res
        return np.concatenate(
            [res.results[c]["xout"].astype(np.float32) for c in range(M)],
            axis=0)
